# revision 21
# baseline (speedup 1.0000x reference)
"""MoE top-1 routing with expert capacity (nn_ExpertAllocation) on 8 TRN2 cores.

Strategy:
- Data-parallel over tokens: 16384 tokens -> 8 shards of 2048.
- Router GEMM: host splits x and W into fp16 hi/lo pieces (x ~= xh + 2^-12*xm)
  and pre-transposes/permutes x so the device streams contiguous fat-descriptor
  DMAs. 3-term fp16 matmul (hH + 2^-12*(hM + mH)) accumulated in fp32 PSUM
  gives better-than-numpy-f32 logits at fp16 PE speed. The (wH|wM) columns are
  packed into PE column groups (tile_position) sharing one moving stream.
- logits^T [64, T] are PE-transposed back to [T(part), 64] tiles for
  softmax/argmax (free-dim reductions).
- One-hot = (logit == rowmax); capacity cumsum over the token dim via
  triangular-ones matmuls + a serial per-tile offset chain; cross-core
  segment offsets via an AllGather of per-core expert counts; aux loss from
  all-gathered count/prob sums.
- Per-group software pipeline: loads(g) | logit-transpose+softmax(g-1) |
  GEMM(g) | cumsum/counts matmuls(g-1), so the PE FIFO never waits on the
  DVE/ACT softmax chains.
"""

import os
import numpy as np
import ml_dtypes

import concourse.bacc as bacc
import concourse.bass as bass
import concourse.mybir as mybir
import concourse.tile as tile
from concourse import bass_utils

F32 = mybir.dt.float32
BF16 = mybir.dt.bfloat16
F16 = mybir.dt.float16
SC = float(2.0 ** -12)          # scale of the fp16 low pieces
FP16_MIN_NORMAL = 6.103515625e-05
AX = mybir.AxisListType
OP = mybir.AluOpType
ACTF = mybir.ActivationFunctionType

B, S, D, E = 4, 4096, 2048, 64
NCORES = 8
TOK = B * S                 # 16384
TPC = TOK // NCORES         # 2048 tokens per core
CAP = float(TOK) / E * 1.0  # 256.0
ALPHA = 0.01
NJ = D // 128               # 16 contraction chunks
NT = TPC // 128             # 16 token tiles per core
NG = 8                      # token groups per core
GT = TPC // NG              # 512 tokens per group
TPG = GT // 128             # 4 token tiles per group


def build_program(single_core=False):
    """single_core=True replaces the collective with a local DMA so the
    program can run under single-core simulators (timing analysis only)."""
    nc = bacc.Bacc("TRN2", target_bir_lowering=False, debug=False,
                   enable_asserts=True,
                   num_devices=1 if single_core else NCORES)

    # x pieces arrive host-permuted as [128(p), NG, NJ, GT]:
    # element [p, g, j, t] = x[g*GT + t, 128*j + p], so each partition row of a
    # per-group load is one contiguous 16 KB run in DRAM.
    xh = nc.dram_tensor("xh", [128, NG, NJ, GT], F16, kind="ExternalInput").ap()
    xm = nc.dram_tensor("xm", [128, NG, NJ, GT], F16, kind="ExternalInput").ap()
    # combined W pieces, host-permuted: [:, j, 0:64]=wH, [:, j, 64:128]=wM
    wHM = nc.dram_tensor("wHM", [128, NJ, 2 * E], F16, kind="ExternalInput").ap()
    bias = nc.dram_tensor("bias", [1, E], F32, kind="ExternalInput").ap()
    ident128 = nc.dram_tensor("ident128", [128, 128], BF16, kind="ExternalInput").ap()
    ident64 = nc.dram_tensor("ident64", [64, 64], F32, kind="ExternalInput").ap()
    onescol = nc.dram_tensor("onescol", [128, 1], F32, kind="ExternalInput").ap()
    prevmask = nc.dram_tensor("prevmask", [NCORES, 1], F32, kind="ExternalInput").ap()

    # out stays in the SBUF-native [128, NT, E] layout; host re-orders
    out = nc.dram_tensor("out", [128, NT, E], F32, kind="ExternalOutput").ap()
    aux = nc.dram_tensor("aux", [1, 1], F32, kind="ExternalOutput").ap()

    cc_in = nc.dram_tensor("cc_in", [2 * E, 1], F32, kind="Internal")
    cc_out = nc.dram_tensor("cc_out", [NCORES, 2 * E], F32, kind="Internal")
    cc_wu_in = nc.dram_tensor("cc_wu_in", [1, 1], F32, kind="Internal")
    cc_wu_out = nc.dram_tensor("cc_wu_out", [NCORES, 1], F32, kind="Internal")

    with tile.TileContext(nc) as tc:
        with tc.tile_pool(name="consts", bufs=1) as consts, \
             tc.tile_pool(name="xt", bufs=3) as xtp, \
             tc.tile_pool(name="work", bufs=3) as work, \
             tc.tile_pool(name="soft", bufs=6) as soft, \
             tc.tile_pool(name="keep", bufs=NG) as keep, \
             tc.tile_pool(name="fin", bufs=1) as finp, \
             tc.tile_pool(name="plog", bufs=2, space="PSUM") as plog, \
             tc.tile_pool(name="psmall", bufs=2, space="PSUM") as psmall, \
             tc.tile_pool(name="pacc", bufs=2, space="PSUM") as pacc:

            # warm up the collective engine early (rings/algo init) so the
            # real AllGather at the end starts without setup latency
            if not single_core:
                wu = consts.tile([1, 1], F32, name="wu")
                nc.vector.memset(wu[:], 0.0)
                nc.sync.dma_start(cc_wu_in.ap(), wu[:])
                nc.gpsimd.collective_compute(
                    "AllGather", OP.bypass,
                    replica_groups=[list(range(NCORES))],
                    ins=[cc_wu_in.ap()], outs=[cc_wu_out.ap()])

            # ---- group-0 x loads first (the DMA critical path) ----
            xth0 = xtp.tile([128, NJ, GT], F16, tag="xth", name="xth")
            nc.sync.dma_start(xth0[:, 0:NJ // 2, :], xh[:, 0, 0:NJ // 2, :])
            nc.sync.dma_start(xth0[:, NJ // 2:, :], xh[:, 0, NJ // 2:, :])
            xtm0 = xtp.tile([128, NJ, GT], F16, tag="xtm", name="xtm")
            nc.sync.dma_start(xtm0[:, 0:NJ // 2, :], xm[:, 0, 0:NJ // 2, :])
            nc.sync.dma_start(xtm0[:, NJ // 2:, :], xm[:, 0, NJ // 2:, :])

            # ---- constants ----
            wHM_sb = consts.tile([128, NJ, 2 * E], F16)
            nc.sync.dma_start(wHM_sb[:], wHM)
            id128_sb = consts.tile([128, 128], BF16)
            nc.sync.dma_start(id128_sb[:], ident128)
            id64_sb = consts.tile([64, 64], F32)
            nc.sync.dma_start(id64_sb[:], ident64)
            ones_c = consts.tile([128, 1], F32)
            nc.sync.dma_start(ones_c[:], onescol)
            pmask = consts.tile([NCORES, 1], F32)
            nc.sync.dma_start(pmask[:], prevmask)
            b1 = consts.tile([1, E], F32)
            nc.sync.dma_start(b1[:], bias)
            bB = consts.tile([128, E], F32)
            nc.gpsimd.partition_broadcast(bB[:], b1[:])

            # transposed one-hot cumsum state: cumT[e, t] = local inclusive
            # per-expert running count (built by per-group free-dim scans)
            cumT = consts.tile([64, TPC], F32)
            scandum = consts.tile([64, GT], F32)
            nc.vector.memset(scandum[:], 0.0)

            # stats column: rows 0:64 = counts^T, rows 64:128 = sum-probs^T
            p_S = pacc.tile([128, 1], F32, tag="pacc")

            ru_k = {}    # per-group routed probs [128, TPG, E]
            state = {}   # per-tile softmax products needed by section B
            ohT_k = {}   # per-group transposed one-hots in PSUM

            def section_a(g, ltB):
                """Logit re-transpose + full softmax/one-hot chain for each of
                group g's four 128-token tiles (PE work is just 4 transposes;
                the rest streams on DVE/ACT while the next GEMM runs)."""
                for i in range(TPG):
                    t = g * TPG + i
                    sl = slice(i * 128, (i + 1) * 128)

                    p_lg = psmall.tile([128, E], F32, tag="psmall", name="p_lg")
                    nc.tensor.transpose(p_lg[:], ltB[:, sl], id64_sb[:])

                    lg = soft.tile([128, E], F32, tag="lg", name="lg")
                    nc.vector.tensor_tensor(lg[:], p_lg[:], bB[:], op=OP.add)

                    m = soft.tile([128, 1], F32, tag="m", name="m")
                    nc.vector.reduce_max(m[:], lg[:], axis=AX.X, negate=True)
                    ex = soft.tile([128, E], F32, tag="ex", name="ex")
                    ssum = soft.tile([128, 1], F32, tag="ssum", name="ssum")
                    nc.scalar.activation(ex[:], lg[:], ACTF.Exp,
                                         bias=m[:], scale=1.0, accum_out=ssum[:])
                    rcp = soft.tile([128, 1], F32, tag="rcp", name="rcp")
                    nc.vector.reciprocal(rcp[:], ssum[:])
                    probs = soft.tile([128, E], F32, tag="probs", name="probs")
                    nc.scalar.mul(probs[:], ex[:], rcp[:])

                    # one-hot of argmax: (logit + (-max)) == 0
                    oh = soft.tile([128, E], BF16, tag="oh", name="oh")
                    nc.vector.tensor_scalar(oh[:], lg[:], m[:], 0.0,
                                            op0=OP.add, op1=OP.is_equal)

                    # routed prob = probs * onehot (kept for phase 3)
                    if i == 0:
                        ru_k[g] = keep.tile([128, TPG, E], F32, tag="ru",
                                            name="ru", bufs=NG)
                        ohT_k[g] = psmall.tile([64, TPG * 128], BF16,
                                               tag="ohT", name="ohT", bufs=1)
                    nc.vector.tensor_tensor(ru_k[g][:, i, :], probs[:],
                                            oh[:], op=OP.mult)
                    # transposed one-hot for the capacity scan
                    nc.tensor.transpose(ohT_k[g][:, i * 128:(i + 1) * 128],
                                        oh[:], id128_sb[:])
                    state[t] = (probs, oh)
                # per-group inclusive scan along tokens, chained via initial
                init = 0.0 if g == 0 else cumT[:, g * GT - 1:g * GT]
                nc.vector.tensor_tensor_scan(
                    cumT[:, g * GT:(g + 1) * GT], ohT_k[g][:],
                    scandum[:], init, op0=OP.add, op1=OP.bypass)

            def section_b(g):
                """Transposed prob-sum accumulation (PE inputs ready by the
                time the FIFO reaches these)."""
                for i in range(TPG):
                    t = g * TPG + i
                    probs, oh = state.pop(t)
                    # p_S[64:128] += probs^T @ ones  (sum over this tile's
                    # tokens, transposed layout)
                    nc.tensor.matmul(p_S[64:128, :], probs[:], ones_c[:],
                                     start=(t == 0), stop=(t == NT - 1))

            prev = None
            for g in range(NG):
                # ---- fat-descriptor loads, split in j-halves ----
                if g == 0:
                    xth, xtm = xth0, xtm0
                else:
                    xth = xtp.tile([128, NJ, GT], F16, tag="xth", name="xth")
                    nc.sync.dma_start(xth[:, 0:NJ // 2, :],
                                      xh[:, g, 0:NJ // 2, :])
                    nc.sync.dma_start(xth[:, NJ // 2:, :],
                                      xh[:, g, NJ // 2:, :])
                    xtm = xtp.tile([128, NJ, GT], F16, tag="xtm", name="xtm")
                    nc.sync.dma_start(xtm[:, 0:NJ // 2, :],
                                      xm[:, g, 0:NJ // 2, :])
                    nc.sync.dma_start(xtm[:, NJ // 2:, :],
                                      xm[:, g, NJ // 2:, :])

                if prev is not None:
                    section_a(prev[0], prev[1])

                # ---- 3-term GEMM: logits = hH + SC*(hM + mH) ----
                # pA = [wH|wM].T @ xh  (rows 0:64 = hH, rows 64:128 = hM)
                # pB = [wH|wM].T @ xm  (rows 0:64 = mH, rows 64:128 unused mM)
                pA = plog.tile([128, GT], F32, tag="pA")
                pB = plog.tile([128, GT], F32, tag="pB", bufs=1)
                for j in range(NJ):
                    nc.tensor.matmul(pA[:, :], wHM_sb[:, j, :], xth[:, j, :],
                                     start=(j == 0), stop=(j == NJ - 1))
                ltA = work.tile([64, GT], F32, tag="ltA")
                nc.scalar.copy(ltA[:], pA[64:128, :])
                for j in range(NJ):
                    nc.tensor.matmul(pB[:, :], wHM_sb[:, j, :],
                                     xtm[:, j, :], start=(j == 0),
                                     stop=(j == NJ - 1))
                # combine: ltB = (ltA + pB_mH)*SC + pA_hH
                ltS = work.tile([64, GT], F32, tag="ltS")
                nc.vector.tensor_tensor(ltS[:], ltA[:], pB[0:64, :], op=OP.add)
                ltB = work.tile([64, GT], F32, tag="ltB")
                nc.vector.scalar_tensor_tensor(ltB[:], ltS[:], SC, pA[0:64, :],
                                               op0=OP.mult, op1=OP.add)
                if prev is not None:
                    section_b(prev[0])
                prev = (g, ltB)
            section_a(prev[0], prev[1])
            section_b(prev[0])

            # ---- cross-core exchange: column [counts^T; probsums^T] ----
            nc.vector.tensor_copy(p_S[0:64, :], cumT[:, TPC - 1:TPC])
            stats = work.tile([128, 1], F32, tag="stats")
            nc.vector.tensor_copy(stats[:], p_S[:])
            nc.sync.dma_start(cc_in.ap(), stats[:])
            if single_core:
                nc.sync.dma_start(cc_out.ap()[0:1, :],
                                  cc_in.ap().rearrange("k one -> one k"))
            else:
                nc.gpsimd.collective_compute(
                    "AllGather", OP.bypass,
                    replica_groups=[list(range(NCORES))],
                    ins=[cc_in.ap()], outs=[cc_out.ap()])
            gath = work.tile([NCORES, 2 * E], F32, tag="gath")
            nc.sync.dma_start(gath[:], cc_out.ap())

            # per-core segment offset (transposed): rows 0:64 of gath^T@mask
            p_off2 = pacc.tile([128, 1], F32, tag="pacc")
            nc.tensor.matmul(p_off2[:], gath[:], pmask[:], start=True,
                             stop=True)
            # capoffT = CAP - coreoff^T  (per-expert threshold column)
            capoffT = work.tile([64, 1], F32, tag="capoffT")
            nc.vector.tensor_scalar(capoffT[:], p_off2[0:64, :], -1.0, CAP,
                                    op0=OP.mult, op1=OP.add)

            # aux loss = ALPHA*E * sum(counts/TOK * probsum/TOK)
            p_tot = pacc.tile([1, 2 * E], F32, tag="pacc")
            nc.tensor.matmul(p_tot[:], ones_c[0:NCORES, :], gath[:],
                             start=True, stop=True)
            tots = work.tile([1, 2 * E], F32, tag="tots")
            nc.vector.tensor_copy(tots[:], p_tot[:])
            fp = work.tile([1, E], F32, tag="fp")
            nc.vector.tensor_tensor(fp[:], tots[0:1, 0:E], tots[0:1, E:2 * E],
                                    op=OP.mult)
            auxv = work.tile([1, 1], F32, tag="auxv")
            nc.vector.reduce_sum(auxv[:], fp[:], axis=AX.X)
            aux_sb = work.tile([1, 1], F32, tag="aux_sb")
            nc.vector.tensor_scalar(aux_sb[:], auxv[:],
                                    float(ALPHA * E / (TOK * TOK)), None,
                                    op0=OP.mult)
            nc.sync.dma_start(aux, aux_sb[:])

            # ---- capacity mask (computed in [e, t], transposed back) ----
            fin = finp.tile([128, NT, E], F32)
            out_r = out
            for g in range(NG):
                mT = work.tile([64, GT], F32, tag="mT")
                nc.vector.tensor_scalar(mT[:], cumT[:, g * GT:(g + 1) * GT],
                                        capoffT[:], None, op0=OP.is_le)
                for i in range(TPG):
                    t = g * TPG + i
                    p_msk = psmall.tile([128, E], F32, tag="psmall",
                                        name="p_msk")
                    nc.tensor.transpose(p_msk[:],
                                        mT[:, i * 128:(i + 1) * 128],
                                        id64_sb[:])
                    nc.vector.tensor_tensor(fin[:, t, :], p_msk[:],
                                            ru_k[g][:, i, :], op=OP.mult)
                nc.sync.dma_start(out_r[:, g * TPG:(g + 1) * TPG, :],
                                  fin[:, g * TPG:(g + 1) * TPG, :])

    nc.compile()
    return nc


_CACHE = {}


def _get_program():
    if "nc" not in _CACHE:
        _CACHE["nc"] = build_program()
    return _CACHE["nc"]


def _split_fp16(a):
    """a ~= ah + 2^-12 * am, both fp16, subnormals flushed host-side."""
    ah = a.astype(np.float16).astype(np.float32)
    ah[np.abs(ah) < FP16_MIN_NORMAL] = 0.0
    ah16 = ah.astype(np.float16)
    am = ((a - ah) * float(2.0 ** 12)).astype(np.float16).astype(np.float32)
    am[np.abs(am) < FP16_MIN_NORMAL] = 0.0
    return ah16, am.astype(np.float16)


def _dev_layout(piece):
    """[TPC, D] fp16 shard piece -> [128, NG, NJ, GT] device layout."""
    return np.ascontiguousarray(
        piece.reshape(NG, GT, NJ, 128).transpose(3, 0, 2, 1))


def _prep_inputs(x, W, b):
    bf = ml_dtypes.bfloat16
    xf = np.ascontiguousarray(np.asarray(x).reshape(TOK, D)).astype(np.float32)
    xh, xm = _split_fp16(xf)
    Wf = np.asarray(W, dtype=np.float32)
    wH, wM = _split_fp16(Wf)
    wH = wH.reshape(NJ, 128, E).transpose(1, 0, 2)
    wM = wM.reshape(NJ, 128, E).transpose(1, 0, 2)
    wHM = np.ascontiguousarray(np.concatenate([wH, wM], axis=2))
    bias = np.asarray(b, dtype=np.float32).reshape(1, E)
    ident128 = np.eye(128, dtype=np.float32).astype(bf)
    ident64 = np.eye(64, dtype=np.float32)
    onescol = np.ones((128, 1), dtype=np.float32)

    in_maps = []
    for c in range(NCORES):
        pm = np.zeros((NCORES, 1), dtype=np.float32)
        pm[:c] = 1.0
        sl = slice(c * TPC, (c + 1) * TPC)
        in_maps.append({
            "xh": _dev_layout(xh[sl]),
            "xm": _dev_layout(xm[sl]),
            "wHM": wHM, "bias": bias, "ident128": ident128,
            "ident64": ident64, "onescol": onescol,
            "prevmask": pm,
        })
    return in_maps


def run(x, W, b, trace=False, trace_cores=None):
    nc = _get_program()
    in_maps = _prep_inputs(x, W, b)
    kw = {}
    if trace_cores is not None:
        kw["trace_cores"] = trace_cores
    res = bass_utils.run_bass_kernel_spmd(
        nc, in_maps, core_ids=list(range(NCORES)), trace=trace, **kw)
    shards = [
        np.transpose(res.results[c]["out"], (1, 0, 2)).reshape(TPC, E)
        for c in range(NCORES)
    ]
    routed = np.concatenate(shards, axis=0).reshape(B, S, E).astype(np.float32)
    aux_loss = np.float32(res.results[0]["aux"][0, 0])
    return (routed, aux_loss), res


def kernel(x, W, b):
    (routed, aux_loss), _ = run(x, W, b, trace=False)
    return routed, aux_loss


# revision 22
# speedup vs baseline: 2.0643x; 2.0643x over previous
"""MoE top-1 routing with expert capacity (nn_ExpertAllocation) on 8 TRN2 cores.

Strategy:
- Data-parallel over tokens: 16384 tokens -> 8 shards of 2048.
- Router GEMM: host splits x and W into fp16 hi/lo pieces (x ~= xh + 2^-12*xm)
  and pre-transposes/permutes x so the device streams contiguous fat-descriptor
  DMAs. 3-term fp16 matmul (hH + 2^-12*(hM + mH)) accumulated in fp32 PSUM
  gives better-than-numpy-f32 logits at fp16 PE speed. The (wH|wM) columns are
  packed into PE column groups (tile_position) sharing one moving stream.
- logits^T [64, T] are PE-transposed back to [T(part), 64] tiles for
  softmax/argmax (free-dim reductions).
- One-hot = (logit == rowmax); capacity cumsum over the token dim via
  triangular-ones matmuls + a serial per-tile offset chain; cross-core
  segment offsets via an AllGather of per-core expert counts; aux loss from
  all-gathered count/prob sums.
- Per-group software pipeline: loads(g) | logit-transpose+softmax(g-1) |
  GEMM(g) | cumsum/counts matmuls(g-1), so the PE FIFO never waits on the
  DVE/ACT softmax chains.
"""

import os
import numpy as np
import ml_dtypes

import concourse.bacc as bacc
import concourse.bass as bass
import concourse.mybir as mybir
import concourse.tile as tile
from concourse import bass_utils

F32 = mybir.dt.float32
BF16 = mybir.dt.bfloat16
F16 = mybir.dt.float16
SC = float(2.0 ** -12)          # scale of the fp16 low pieces
FP16_MIN_NORMAL = 6.103515625e-05
AX = mybir.AxisListType
OP = mybir.AluOpType
ACTF = mybir.ActivationFunctionType

B, S, D, E = 4, 4096, 2048, 64
NCORES = 8
TOK = B * S                 # 16384
TPC = TOK // NCORES         # 2048 tokens per core
CAP = float(TOK) / E * 1.0  # 256.0
ALPHA = 0.01
NJ = D // 128               # 16 contraction chunks
NT = TPC // 128             # 16 token tiles per core
NG = 8                      # token groups per core
GT = TPC // NG              # 512 tokens per group
TPG = GT // 128             # 4 token tiles per group


def build_program(single_core=False):
    """single_core=True replaces the collective with a local DMA so the
    program can run under single-core simulators (timing analysis only)."""
    nc = bacc.Bacc("TRN2", target_bir_lowering=False, debug=False,
                   enable_asserts=True,
                   num_devices=1 if single_core else NCORES)

    # x pieces arrive host-permuted as [128(p), NG, NJ, GT]:
    # element [p, g, j, t] = x[g*GT + t, 128*j + p], so each partition row of a
    # per-group load is one contiguous 16 KB run in DRAM.
    xh = nc.dram_tensor("xh", [128, NG, NJ, GT], F16, kind="ExternalInput").ap()
    xm = nc.dram_tensor("xm", [128, NG, NJ, GT], F16, kind="ExternalInput").ap()
    # combined W pieces, host-permuted: [:, j, 0:64]=wH, [:, j, 64:128]=wM
    wHM = nc.dram_tensor("wHM", [128, NJ, 2 * E], F16, kind="ExternalInput").ap()
    bias = nc.dram_tensor("bias", [1, E], F32, kind="ExternalInput").ap()
    ident128 = nc.dram_tensor("ident128", [128, 128], BF16, kind="ExternalInput").ap()
    ident64 = nc.dram_tensor("ident64", [64, 64], F32, kind="ExternalInput").ap()
    onescol = nc.dram_tensor("onescol", [128, 1], F32, kind="ExternalInput").ap()
    prevmask = nc.dram_tensor("prevmask", [NCORES, 1], F32, kind="ExternalInput").ap()

    # out stays in the SBUF-native [128, NT, E] layout; host re-orders
    out = nc.dram_tensor("out", [128, NT, E], F32, kind="ExternalOutput").ap()
    aux = nc.dram_tensor("aux", [1, 1], F32, kind="ExternalOutput").ap()

    cc_in = nc.dram_tensor("cc_in", [2 * E, 1], F32, kind="Internal")
    cc_out = nc.dram_tensor("cc_out", [NCORES, 2 * E], F32, kind="Internal")
    cc_wu_in = nc.dram_tensor("cc_wu_in", [1, 1], F32, kind="Internal")
    cc_wu_out = nc.dram_tensor("cc_wu_out", [NCORES, 1], F32, kind="Internal")

    with tile.TileContext(nc) as tc:
        with tc.tile_pool(name="consts", bufs=1) as consts, \
             tc.tile_pool(name="xt", bufs=3) as xtp, \
             tc.tile_pool(name="work", bufs=3) as work, \
             tc.tile_pool(name="soft", bufs=6) as soft, \
             tc.tile_pool(name="keep", bufs=NG) as keep, \
             tc.tile_pool(name="fin", bufs=1) as finp, \
             tc.tile_pool(name="plog", bufs=2, space="PSUM") as plog, \
             tc.tile_pool(name="psmall", bufs=2, space="PSUM") as psmall, \
             tc.tile_pool(name="pacc", bufs=2, space="PSUM") as pacc:

            # warm up the collective engine early (rings/algo init) so the
            # real AllGather at the end starts without setup latency
            if not single_core:
                wu = consts.tile([1, 1], F32, name="wu")
                nc.vector.memset(wu[:], 0.0)
                nc.sync.dma_start(cc_wu_in.ap(), wu[:])
                nc.gpsimd.collective_compute(
                    "AllGather", OP.bypass,
                    replica_groups=[list(range(NCORES))],
                    ins=[cc_wu_in.ap()], outs=[cc_wu_out.ap()])

            # ---- group-0 x loads first (the DMA critical path) ----
            xth0 = xtp.tile([128, NJ, GT], F16, tag="xth", name="xth")
            nc.sync.dma_start(xth0[:, 0:NJ // 2, :], xh[:, 0, 0:NJ // 2, :])
            nc.sync.dma_start(xth0[:, NJ // 2:, :], xh[:, 0, NJ // 2:, :])
            xtm0 = xtp.tile([128, NJ, GT], F16, tag="xtm", name="xtm")
            nc.scalar.dma_start(xtm0[:, 0:NJ // 2, :], xm[:, 0, 0:NJ // 2, :])
            nc.scalar.dma_start(xtm0[:, NJ // 2:, :], xm[:, 0, NJ // 2:, :])

            # ---- constants ----
            wHM_sb = consts.tile([128, NJ, 2 * E], F16)
            nc.sync.dma_start(wHM_sb[:], wHM)
            id128_sb = consts.tile([128, 128], BF16)
            nc.sync.dma_start(id128_sb[:], ident128)
            id64_sb = consts.tile([64, 64], F32)
            nc.sync.dma_start(id64_sb[:], ident64)
            ones_c = consts.tile([128, 1], F32)
            nc.sync.dma_start(ones_c[:], onescol)
            pmask = consts.tile([NCORES, 1], F32)
            nc.sync.dma_start(pmask[:], prevmask)
            b1 = consts.tile([1, E], F32)
            nc.sync.dma_start(b1[:], bias)
            bB = consts.tile([128, E], F32)
            nc.gpsimd.partition_broadcast(bB[:], b1[:])

            # transposed one-hot cumsum state: cumT[e, t] = local inclusive
            # per-expert running count (built by per-group free-dim scans)
            cumT = consts.tile([64, TPC], F32)
            scandum = consts.tile([64, GT], F32)
            nc.vector.memset(scandum[:], 0.0)

            # stats column: rows 0:64 = counts^T, rows 64:128 = sum-probs^T
            p_S = pacc.tile([128, 1], F32, tag="pacc")

            ru_k = {}    # per-group routed probs [128, TPG, E]
            state = {}   # per-tile softmax products needed by section B
            ohT_k = {}   # per-group transposed one-hots in PSUM

            def section_a(g, ltB):
                """Logit re-transpose + full softmax/one-hot chain for each of
                group g's four 128-token tiles (PE work is just 4 transposes;
                the rest streams on DVE/ACT while the next GEMM runs)."""
                for i in range(TPG):
                    t = g * TPG + i
                    sl = slice(i * 128, (i + 1) * 128)

                    p_lg = psmall.tile([128, E], F32, tag="psmall", name="p_lg")
                    nc.tensor.transpose(p_lg[:], ltB[:, sl], id64_sb[:])

                    lg = soft.tile([128, E], F32, tag="lg", name="lg")
                    nc.vector.tensor_tensor(lg[:], p_lg[:], bB[:], op=OP.add)

                    m = soft.tile([128, 1], F32, tag="m", name="m")
                    nc.vector.reduce_max(m[:], lg[:], axis=AX.X, negate=True)
                    ex = soft.tile([128, E], F32, tag="ex", name="ex")
                    ssum = soft.tile([128, 1], F32, tag="ssum", name="ssum")
                    nc.scalar.activation(ex[:], lg[:], ACTF.Exp,
                                         bias=m[:], scale=1.0, accum_out=ssum[:])
                    rcp = soft.tile([128, 1], F32, tag="rcp", name="rcp")
                    nc.vector.reciprocal(rcp[:], ssum[:])
                    probs = soft.tile([128, E], F32, tag="probs", name="probs")
                    nc.scalar.mul(probs[:], ex[:], rcp[:])

                    # one-hot of argmax: (logit + (-max)) == 0
                    oh = soft.tile([128, E], BF16, tag="oh", name="oh")
                    nc.vector.tensor_scalar(oh[:], lg[:], m[:], 0.0,
                                            op0=OP.add, op1=OP.is_equal)

                    # routed prob = probs * onehot (kept for phase 3)
                    if i == 0:
                        ru_k[g] = keep.tile([128, TPG, E], F32, tag="ru",
                                            name="ru", bufs=NG)
                        ohT_k[g] = psmall.tile([64, TPG * 128], BF16,
                                               tag="ohT", name="ohT", bufs=1)
                    nc.vector.tensor_tensor(ru_k[g][:, i, :], probs[:],
                                            oh[:], op=OP.mult)
                    # transposed one-hot for the capacity scan
                    nc.tensor.transpose(ohT_k[g][:, i * 128:(i + 1) * 128],
                                        oh[:], id128_sb[:])
                    state[t] = (probs, oh)
                # per-group inclusive scan along tokens, chained via initial
                init = 0.0 if g == 0 else cumT[:, g * GT - 1:g * GT]
                nc.vector.tensor_tensor_scan(
                    cumT[:, g * GT:(g + 1) * GT], ohT_k[g][:],
                    scandum[:], init, op0=OP.add, op1=OP.bypass)

            def section_b(g):
                """Transposed prob-sum accumulation (PE inputs ready by the
                time the FIFO reaches these)."""
                for i in range(TPG):
                    t = g * TPG + i
                    probs, oh = state.pop(t)
                    # p_S[64:128] += probs^T @ ones  (sum over this tile's
                    # tokens, transposed layout)
                    nc.tensor.matmul(p_S[64:128, :], probs[:], ones_c[:],
                                     start=(t == 0), stop=(t == NT - 1))

            prev = None
            for g in range(NG):
                # ---- fat-descriptor loads, split in j-halves ----
                if g == 0:
                    xth, xtm = xth0, xtm0
                else:
                    xth = xtp.tile([128, NJ, GT], F16, tag="xth", name="xth")
                    nc.sync.dma_start(xth[:, 0:NJ // 2, :],
                                      xh[:, g, 0:NJ // 2, :])
                    nc.sync.dma_start(xth[:, NJ // 2:, :],
                                      xh[:, g, NJ // 2:, :])
                    xtm = xtp.tile([128, NJ, GT], F16, tag="xtm", name="xtm")
                    nc.scalar.dma_start(xtm[:, 0:NJ // 2, :],
                                      xm[:, g, 0:NJ // 2, :])
                    nc.scalar.dma_start(xtm[:, NJ // 2:, :],
                                      xm[:, g, NJ // 2:, :])

                if prev is not None:
                    section_a(prev[0], prev[1])

                # ---- 3-term GEMM: logits = hH + SC*(hM + mH) ----
                # pA = [wH|wM].T @ xh  (rows 0:64 = hH, rows 64:128 = hM)
                # pB = [wH|wM].T @ xm  (rows 0:64 = mH, rows 64:128 unused mM)
                pA = plog.tile([128, GT], F32, tag="pA")
                pB = plog.tile([128, GT], F32, tag="pB", bufs=1)
                for j in range(NJ):
                    nc.tensor.matmul(pA[:, :], wHM_sb[:, j, :], xth[:, j, :],
                                     start=(j == 0), stop=(j == NJ - 1))
                ltA = work.tile([64, GT], F32, tag="ltA")
                nc.scalar.copy(ltA[:], pA[64:128, :])
                for j in range(NJ):
                    nc.tensor.matmul(pB[:, :], wHM_sb[:, j, :],
                                     xtm[:, j, :], start=(j == 0),
                                     stop=(j == NJ - 1))
                # combine: ltB = (ltA + pB_mH)*SC + pA_hH
                ltS = work.tile([64, GT], F32, tag="ltS")
                nc.vector.tensor_tensor(ltS[:], ltA[:], pB[0:64, :], op=OP.add)
                ltB = work.tile([64, GT], F32, tag="ltB")
                nc.vector.scalar_tensor_tensor(ltB[:], ltS[:], SC, pA[0:64, :],
                                               op0=OP.mult, op1=OP.add)
                if prev is not None:
                    section_b(prev[0])
                prev = (g, ltB)
            section_a(prev[0], prev[1])
            section_b(prev[0])

            # ---- cross-core exchange: column [counts^T; probsums^T] ----
            nc.vector.tensor_copy(p_S[0:64, :], cumT[:, TPC - 1:TPC])
            stats = work.tile([128, 1], F32, tag="stats")
            nc.vector.tensor_copy(stats[:], p_S[:])
            nc.sync.dma_start(cc_in.ap(), stats[:])
            if single_core:
                nc.sync.dma_start(cc_out.ap()[0:1, :],
                                  cc_in.ap().rearrange("k one -> one k"))
            else:
                nc.gpsimd.collective_compute(
                    "AllGather", OP.bypass,
                    replica_groups=[list(range(NCORES))],
                    ins=[cc_in.ap()], outs=[cc_out.ap()])
            gath = work.tile([NCORES, 2 * E], F32, tag="gath")
            nc.sync.dma_start(gath[:], cc_out.ap())

            # per-core segment offset (transposed): rows 0:64 of gath^T@mask
            p_off2 = pacc.tile([128, 1], F32, tag="pacc")
            nc.tensor.matmul(p_off2[:], gath[:], pmask[:], start=True,
                             stop=True)
            # capoffT = CAP - coreoff^T  (per-expert threshold column)
            capoffT = work.tile([64, 1], F32, tag="capoffT")
            nc.vector.tensor_scalar(capoffT[:], p_off2[0:64, :], -1.0, CAP,
                                    op0=OP.mult, op1=OP.add)

            # aux loss = ALPHA*E * sum(counts/TOK * probsum/TOK)
            p_tot = pacc.tile([1, 2 * E], F32, tag="pacc")
            nc.tensor.matmul(p_tot[:], ones_c[0:NCORES, :], gath[:],
                             start=True, stop=True)
            tots = work.tile([1, 2 * E], F32, tag="tots")
            nc.vector.tensor_copy(tots[:], p_tot[:])
            fp = work.tile([1, E], F32, tag="fp")
            nc.vector.tensor_tensor(fp[:], tots[0:1, 0:E], tots[0:1, E:2 * E],
                                    op=OP.mult)
            auxv = work.tile([1, 1], F32, tag="auxv")
            nc.vector.reduce_sum(auxv[:], fp[:], axis=AX.X)
            aux_sb = work.tile([1, 1], F32, tag="aux_sb")
            nc.vector.tensor_scalar(aux_sb[:], auxv[:],
                                    float(ALPHA * E / (TOK * TOK)), None,
                                    op0=OP.mult)
            nc.sync.dma_start(aux, aux_sb[:])

            # ---- capacity mask (computed in [e, t], transposed back) ----
            fin = finp.tile([128, NT, E], F32)
            out_r = out
            for g in range(NG):
                mT = work.tile([64, GT], F32, tag="mT")
                nc.vector.tensor_scalar(mT[:], cumT[:, g * GT:(g + 1) * GT],
                                        capoffT[:], None, op0=OP.is_le)
                for i in range(TPG):
                    t = g * TPG + i
                    p_msk = psmall.tile([128, E], F32, tag="psmall",
                                        name="p_msk")
                    nc.tensor.transpose(p_msk[:],
                                        mT[:, i * 128:(i + 1) * 128],
                                        id64_sb[:])
                    nc.vector.tensor_tensor(fin[:, t, :], p_msk[:],
                                            ru_k[g][:, i, :], op=OP.mult)
                nc.sync.dma_start(out_r[:, g * TPG:(g + 1) * TPG, :],
                                  fin[:, g * TPG:(g + 1) * TPG, :])

    nc.compile()
    return nc


_CACHE = {}


def _get_program():
    if "nc" not in _CACHE:
        _CACHE["nc"] = build_program()
    return _CACHE["nc"]


def _split_fp16(a):
    """a ~= ah + 2^-12 * am, both fp16, subnormals flushed host-side."""
    ah = a.astype(np.float16).astype(np.float32)
    ah[np.abs(ah) < FP16_MIN_NORMAL] = 0.0
    ah16 = ah.astype(np.float16)
    am = ((a - ah) * float(2.0 ** 12)).astype(np.float16).astype(np.float32)
    am[np.abs(am) < FP16_MIN_NORMAL] = 0.0
    return ah16, am.astype(np.float16)


def _dev_layout(piece):
    """[TPC, D] fp16 shard piece -> [128, NG, NJ, GT] device layout."""
    return np.ascontiguousarray(
        piece.reshape(NG, GT, NJ, 128).transpose(3, 0, 2, 1))


def _prep_inputs(x, W, b):
    bf = ml_dtypes.bfloat16
    xf = np.ascontiguousarray(np.asarray(x).reshape(TOK, D)).astype(np.float32)
    xh, xm = _split_fp16(xf)
    Wf = np.asarray(W, dtype=np.float32)
    wH, wM = _split_fp16(Wf)
    wH = wH.reshape(NJ, 128, E).transpose(1, 0, 2)
    wM = wM.reshape(NJ, 128, E).transpose(1, 0, 2)
    wHM = np.ascontiguousarray(np.concatenate([wH, wM], axis=2))
    bias = np.asarray(b, dtype=np.float32).reshape(1, E)
    ident128 = np.eye(128, dtype=np.float32).astype(bf)
    ident64 = np.eye(64, dtype=np.float32)
    onescol = np.ones((128, 1), dtype=np.float32)

    in_maps = []
    for c in range(NCORES):
        pm = np.zeros((NCORES, 1), dtype=np.float32)
        pm[:c] = 1.0
        sl = slice(c * TPC, (c + 1) * TPC)
        in_maps.append({
            "xh": _dev_layout(xh[sl]),
            "xm": _dev_layout(xm[sl]),
            "wHM": wHM, "bias": bias, "ident128": ident128,
            "ident64": ident64, "onescol": onescol,
            "prevmask": pm,
        })
    return in_maps


def run(x, W, b, trace=False, trace_cores=None):
    nc = _get_program()
    in_maps = _prep_inputs(x, W, b)
    kw = {}
    if trace_cores is not None:
        kw["trace_cores"] = trace_cores
    res = bass_utils.run_bass_kernel_spmd(
        nc, in_maps, core_ids=list(range(NCORES)), trace=trace, **kw)
    shards = [
        np.transpose(res.results[c]["out"], (1, 0, 2)).reshape(TPC, E)
        for c in range(NCORES)
    ]
    routed = np.concatenate(shards, axis=0).reshape(B, S, E).astype(np.float32)
    aux_loss = np.float32(res.results[0]["aux"][0, 0])
    return (routed, aux_loss), res


def kernel(x, W, b):
    (routed, aux_loss), _ = run(x, W, b, trace=False)
    return routed, aux_loss


# revision 23
# speedup vs baseline: 2.0702x; 1.0028x over previous
"""MoE top-1 routing with expert capacity (nn_ExpertAllocation) on 8 TRN2 cores.

Strategy:
- Data-parallel over tokens: 16384 tokens -> 8 shards of 2048.
- Router GEMM: host splits x and W into fp16 hi/lo pieces (x ~= xh + 2^-12*xm)
  and pre-transposes/permutes x so the device streams contiguous fat-descriptor
  DMAs. 3-term fp16 matmul (hH + 2^-12*(hM + mH)) accumulated in fp32 PSUM
  gives better-than-numpy-f32 logits at fp16 PE speed. The (wH|wM) columns are
  packed into PE column groups (tile_position) sharing one moving stream.
- logits^T [64, T] are PE-transposed back to [T(part), 64] tiles for
  softmax/argmax (free-dim reductions).
- One-hot = (logit == rowmax); capacity cumsum over the token dim via
  triangular-ones matmuls + a serial per-tile offset chain; cross-core
  segment offsets via an AllGather of per-core expert counts; aux loss from
  all-gathered count/prob sums.
- Per-group software pipeline: loads(g) | logit-transpose+softmax(g-1) |
  GEMM(g) | cumsum/counts matmuls(g-1), so the PE FIFO never waits on the
  DVE/ACT softmax chains.
"""

import os
import numpy as np
import ml_dtypes

import concourse.bacc as bacc
import concourse.bass as bass
import concourse.mybir as mybir
import concourse.tile as tile
from concourse import bass_utils

F32 = mybir.dt.float32
BF16 = mybir.dt.bfloat16
F16 = mybir.dt.float16
SC = float(2.0 ** -12)          # scale of the fp16 low pieces
FP16_MIN_NORMAL = 6.103515625e-05
AX = mybir.AxisListType
OP = mybir.AluOpType
ACTF = mybir.ActivationFunctionType

B, S, D, E = 4, 4096, 2048, 64
NCORES = 8
TOK = B * S                 # 16384
TPC = TOK // NCORES         # 2048 tokens per core
CAP = float(TOK) / E * 1.0  # 256.0
ALPHA = 0.01
NJ = D // 128               # 16 contraction chunks
NT = TPC // 128             # 16 token tiles per core
NG = 8                      # token groups per core
GT = TPC // NG              # 512 tokens per group
TPG = GT // 128             # 4 token tiles per group


def build_program(single_core=False):
    """single_core=True replaces the collective with a local DMA so the
    program can run under single-core simulators (timing analysis only)."""
    nc = bacc.Bacc("TRN2", target_bir_lowering=False, debug=False,
                   enable_asserts=True,
                   num_devices=1 if single_core else NCORES)

    # x pieces arrive host-permuted as [128(p), NG, NJ, GT]:
    # element [p, g, j, t] = x[g*GT + t, 128*j + p], so each partition row of a
    # per-group load is one contiguous 16 KB run in DRAM.
    xh = nc.dram_tensor("xh", [128, NG, NJ, GT], F16, kind="ExternalInput").ap()
    xm = nc.dram_tensor("xm", [128, NG, NJ, GT], F16, kind="ExternalInput").ap()
    # combined W pieces, host-permuted: [:, j, 0:64]=wH, [:, j, 64:128]=wM
    wHM = nc.dram_tensor("wHM", [128, NJ, 2 * E], F16, kind="ExternalInput").ap()
    bias = nc.dram_tensor("bias", [1, E], F32, kind="ExternalInput").ap()
    ident128 = nc.dram_tensor("ident128", [128, 128], BF16, kind="ExternalInput").ap()
    ident64 = nc.dram_tensor("ident64", [64, 64], F32, kind="ExternalInput").ap()
    onescol = nc.dram_tensor("onescol", [128, 1], F32, kind="ExternalInput").ap()
    prevmask = nc.dram_tensor("prevmask", [NCORES, 1], F32, kind="ExternalInput").ap()

    # out stays in the SBUF-native [128, NT, E] layout; host re-orders
    out = nc.dram_tensor("out", [128, NT, E], F32, kind="ExternalOutput").ap()
    aux = nc.dram_tensor("aux", [1, 1], F32, kind="ExternalOutput").ap()

    cc_in = nc.dram_tensor("cc_in", [2 * E, 1], F32, kind="Internal")
    cc_out = nc.dram_tensor("cc_out", [NCORES, 2 * E], F32, kind="Internal")
    cc_wu_in = nc.dram_tensor("cc_wu_in", [1, 1], F32, kind="Internal")
    cc_wu_out = nc.dram_tensor("cc_wu_out", [NCORES, 1], F32, kind="Internal")

    with tile.TileContext(nc) as tc:
        with tc.tile_pool(name="consts", bufs=1) as consts, \
             tc.tile_pool(name="xt", bufs=3) as xtp, \
             tc.tile_pool(name="work", bufs=3) as work, \
             tc.tile_pool(name="soft", bufs=6) as soft, \
             tc.tile_pool(name="keep", bufs=NG) as keep, \
             tc.tile_pool(name="fin", bufs=1) as finp, \
             tc.tile_pool(name="plog", bufs=2, space="PSUM") as plog, \
             tc.tile_pool(name="psmall", bufs=2, space="PSUM") as psmall, \
             tc.tile_pool(name="pacc", bufs=2, space="PSUM") as pacc:

            # warm up the collective engine early (rings/algo init) so the
            # real AllGather at the end starts without setup latency
            if not single_core:
                wu = consts.tile([1, 1], F32, name="wu")
                nc.vector.memset(wu[:], 0.0)
                nc.sync.dma_start(cc_wu_in.ap(), wu[:])
                nc.gpsimd.collective_compute(
                    "AllGather", OP.bypass,
                    replica_groups=[list(range(NCORES))],
                    ins=[cc_wu_in.ap()], outs=[cc_wu_out.ap()])

            # ---- group-0 x loads first (the DMA critical path) ----
            xth0 = xtp.tile([128, NJ, GT], F16, tag="xth", name="xth")
            nc.sync.dma_start(xth0[:, 0:NJ // 2, :], xh[:, 0, 0:NJ // 2, :])
            nc.sync.dma_start(xth0[:, NJ // 2:, :], xh[:, 0, NJ // 2:, :])
            xtm0 = xtp.tile([128, NJ, GT], F16, tag="xtm", name="xtm")
            nc.sync.dma_start(xtm0[:, 0:NJ // 2, :], xm[:, 0, 0:NJ // 2, :])
            nc.sync.dma_start(xtm0[:, NJ // 2:, :], xm[:, 0, NJ // 2:, :])

            # ---- constants ----
            wHM_sb = consts.tile([128, NJ, 2 * E], F16)
            nc.sync.dma_start(wHM_sb[:], wHM)
            id128_sb = consts.tile([128, 128], BF16)
            nc.sync.dma_start(id128_sb[:], ident128)
            id64_sb = consts.tile([64, 64], F32)
            nc.sync.dma_start(id64_sb[:], ident64)
            ones_c = consts.tile([128, 1], F32)
            nc.sync.dma_start(ones_c[:], onescol)
            pmask = consts.tile([NCORES, 1], F32)
            nc.sync.dma_start(pmask[:], prevmask)
            b1 = consts.tile([1, E], F32)
            nc.sync.dma_start(b1[:], bias)
            bB = consts.tile([128, E], F32)
            nc.gpsimd.partition_broadcast(bB[:], b1[:])

            # transposed one-hot cumsum state: cumT[e, t] = local inclusive
            # per-expert running count (built by per-group free-dim scans)
            cumT = consts.tile([64, TPC], F32)
            scandum = consts.tile([64, GT], F32)
            nc.vector.memset(scandum[:], 0.0)

            # stats column: rows 0:64 = counts^T, rows 64:128 = sum-probs^T
            p_S = pacc.tile([128, 1], F32, tag="pacc")

            ru_k = {}    # per-group routed probs [128, TPG, E]
            state = {}   # per-tile softmax products needed by section B
            ohT_k = {}   # per-group transposed one-hots in PSUM

            def section_a(g, ltB):
                """Logit re-transpose + full softmax/one-hot chain for each of
                group g's four 128-token tiles (PE work is just 4 transposes;
                the rest streams on DVE/ACT while the next GEMM runs)."""
                for i in range(TPG):
                    t = g * TPG + i
                    sl = slice(i * 128, (i + 1) * 128)

                    p_lg = psmall.tile([128, E], F32, tag="psmall", name="p_lg")
                    nc.tensor.transpose(p_lg[:], ltB[:, sl], id64_sb[:])

                    lg = soft.tile([128, E], F32, tag="lg", name="lg")
                    nc.vector.tensor_tensor(lg[:], p_lg[:], bB[:], op=OP.add)

                    m = soft.tile([128, 1], F32, tag="m", name="m")
                    nc.vector.reduce_max(m[:], lg[:], axis=AX.X, negate=True)
                    ex = soft.tile([128, E], F32, tag="ex", name="ex")
                    ssum = soft.tile([128, 1], F32, tag="ssum", name="ssum")
                    nc.scalar.activation(ex[:], lg[:], ACTF.Exp,
                                         bias=m[:], scale=1.0, accum_out=ssum[:])
                    rcp = soft.tile([128, 1], F32, tag="rcp", name="rcp")
                    nc.vector.reciprocal(rcp[:], ssum[:])
                    probs = soft.tile([128, E], F32, tag="probs", name="probs")
                    nc.scalar.mul(probs[:], ex[:], rcp[:])

                    # one-hot of argmax: (logit + (-max)) == 0
                    oh = soft.tile([128, E], BF16, tag="oh", name="oh")
                    nc.vector.tensor_scalar(oh[:], lg[:], m[:], 0.0,
                                            op0=OP.add, op1=OP.is_equal)

                    # routed prob = probs * onehot (kept for phase 3)
                    if i == 0:
                        ru_k[g] = keep.tile([128, TPG, E], F32, tag="ru",
                                            name="ru", bufs=NG)
                        ohT_k[g] = psmall.tile([64, TPG * 128], BF16,
                                               tag="ohT", name="ohT", bufs=1)
                    nc.vector.tensor_tensor(ru_k[g][:, i, :], probs[:],
                                            oh[:], op=OP.mult)
                    # transposed one-hot for the capacity scan
                    nc.tensor.transpose(ohT_k[g][:, i * 128:(i + 1) * 128],
                                        oh[:], id128_sb[:])
                    state[t] = (probs, oh)
                # per-group inclusive scan along tokens, chained via initial
                init = 0.0 if g == 0 else cumT[:, g * GT - 1:g * GT]
                nc.vector.tensor_tensor_scan(
                    cumT[:, g * GT:(g + 1) * GT], ohT_k[g][:],
                    scandum[:], init, op0=OP.add, op1=OP.bypass)

            def section_b(g):
                """Transposed prob-sum accumulation (PE inputs ready by the
                time the FIFO reaches these)."""
                for i in range(TPG):
                    t = g * TPG + i
                    probs, oh = state.pop(t)
                    # p_S[64:128] += probs^T @ ones  (sum over this tile's
                    # tokens, transposed layout)
                    nc.tensor.matmul(p_S[64:128, :], probs[:], ones_c[:],
                                     start=(t == 0), stop=(t == NT - 1))

            prev = None
            for g in range(NG):
                # ---- fat-descriptor loads, split in j-halves ----
                if g == 0:
                    xth, xtm = xth0, xtm0
                else:
                    xth = xtp.tile([128, NJ, GT], F16, tag="xth", name="xth")
                    nc.sync.dma_start(xth[:, 0:NJ // 2, :],
                                      xh[:, g, 0:NJ // 2, :])
                    nc.sync.dma_start(xth[:, NJ // 2:, :],
                                      xh[:, g, NJ // 2:, :])
                    xtm = xtp.tile([128, NJ, GT], F16, tag="xtm", name="xtm")
                    nc.sync.dma_start(xtm[:, 0:NJ // 2, :],
                                      xm[:, g, 0:NJ // 2, :])
                    nc.sync.dma_start(xtm[:, NJ // 2:, :],
                                      xm[:, g, NJ // 2:, :])

                if prev is not None:
                    section_a(prev[0], prev[1])

                # ---- 3-term GEMM: logits = hH + SC*(hM + mH) ----
                # pA = [wH|wM].T @ xh  (rows 0:64 = hH, rows 64:128 = hM)
                # pB = [wH|wM].T @ xm  (rows 0:64 = mH, rows 64:128 unused mM)
                pA = plog.tile([128, GT], F32, tag="pA")
                pB = plog.tile([128, GT], F32, tag="pB", bufs=1)
                for j in range(NJ):
                    nc.tensor.matmul(pA[:, :], wHM_sb[:, j, :], xth[:, j, :],
                                     start=(j == 0), stop=(j == NJ - 1))
                ltA = work.tile([64, GT], F32, tag="ltA")
                nc.scalar.copy(ltA[:], pA[64:128, :])
                for j in range(NJ):
                    nc.tensor.matmul(pB[:, :], wHM_sb[:, j, :],
                                     xtm[:, j, :], start=(j == 0),
                                     stop=(j == NJ - 1))
                # combine: ltB = (ltA + pB_mH)*SC + pA_hH
                ltS = work.tile([64, GT], F32, tag="ltS")
                nc.vector.tensor_tensor(ltS[:], ltA[:], pB[0:64, :], op=OP.add)
                ltB = work.tile([64, GT], F32, tag="ltB")
                nc.vector.scalar_tensor_tensor(ltB[:], ltS[:], SC, pA[0:64, :],
                                               op0=OP.mult, op1=OP.add)
                if prev is not None:
                    section_b(prev[0])
                prev = (g, ltB)
            section_a(prev[0], prev[1])
            section_b(prev[0])

            # ---- cross-core exchange: column [counts^T; probsums^T] ----
            nc.vector.tensor_copy(p_S[0:64, :], cumT[:, TPC - 1:TPC])
            stats = work.tile([128, 1], F32, tag="stats")
            nc.vector.tensor_copy(stats[:], p_S[:])
            nc.sync.dma_start(cc_in.ap(), stats[:])
            if single_core:
                nc.sync.dma_start(cc_out.ap()[0:1, :],
                                  cc_in.ap().rearrange("k one -> one k"))
            else:
                nc.gpsimd.collective_compute(
                    "AllGather", OP.bypass,
                    replica_groups=[list(range(NCORES))],
                    ins=[cc_in.ap()], outs=[cc_out.ap()])
            gath = work.tile([NCORES, 2 * E], F32, tag="gath")
            nc.sync.dma_start(gath[:], cc_out.ap())

            # per-core segment offset (transposed): rows 0:64 of gath^T@mask
            p_off2 = pacc.tile([128, 1], F32, tag="pacc")
            nc.tensor.matmul(p_off2[:], gath[:], pmask[:], start=True,
                             stop=True)
            # capoffT = CAP - coreoff^T  (per-expert threshold column)
            capoffT = work.tile([64, 1], F32, tag="capoffT")
            nc.vector.tensor_scalar(capoffT[:], p_off2[0:64, :], -1.0, CAP,
                                    op0=OP.mult, op1=OP.add)

            # aux loss = ALPHA*E * sum(counts/TOK * probsum/TOK)
            p_tot = pacc.tile([1, 2 * E], F32, tag="pacc")
            nc.tensor.matmul(p_tot[:], ones_c[0:NCORES, :], gath[:],
                             start=True, stop=True)
            tots = work.tile([1, 2 * E], F32, tag="tots")
            nc.vector.tensor_copy(tots[:], p_tot[:])
            fp = work.tile([1, E], F32, tag="fp")
            nc.vector.tensor_tensor(fp[:], tots[0:1, 0:E], tots[0:1, E:2 * E],
                                    op=OP.mult)
            auxv = work.tile([1, 1], F32, tag="auxv")
            nc.vector.reduce_sum(auxv[:], fp[:], axis=AX.X)
            aux_sb = work.tile([1, 1], F32, tag="aux_sb")
            nc.vector.tensor_scalar(aux_sb[:], auxv[:],
                                    float(ALPHA * E / (TOK * TOK)), None,
                                    op0=OP.mult)
            nc.sync.dma_start(aux, aux_sb[:])

            # ---- capacity mask (computed in [e, t], transposed back) ----
            fin = finp.tile([128, NT, E], F32)
            out_r = out
            for g in range(NG):
                mT = work.tile([64, GT], F32, tag="mT")
                nc.vector.tensor_scalar(mT[:], cumT[:, g * GT:(g + 1) * GT],
                                        capoffT[:], None, op0=OP.is_le)
                for i in range(TPG):
                    t = g * TPG + i
                    p_msk = psmall.tile([128, E], F32, tag="psmall",
                                        name="p_msk")
                    nc.tensor.transpose(p_msk[:],
                                        mT[:, i * 128:(i + 1) * 128],
                                        id64_sb[:])
                    nc.vector.tensor_tensor(fin[:, t, :], p_msk[:],
                                            ru_k[g][:, i, :], op=OP.mult)
                nc.sync.dma_start(out_r[:, g * TPG:(g + 1) * TPG, :],
                                  fin[:, g * TPG:(g + 1) * TPG, :])

    nc.compile()
    return nc


_CACHE = {}


def _get_program():
    if "nc" not in _CACHE:
        _CACHE["nc"] = build_program()
    return _CACHE["nc"]


def _split_fp16(a):
    """a ~= ah + 2^-12 * am, both fp16, subnormals flushed host-side."""
    ah = a.astype(np.float16).astype(np.float32)
    ah[np.abs(ah) < FP16_MIN_NORMAL] = 0.0
    ah16 = ah.astype(np.float16)
    am = ((a - ah) * float(2.0 ** 12)).astype(np.float16).astype(np.float32)
    am[np.abs(am) < FP16_MIN_NORMAL] = 0.0
    return ah16, am.astype(np.float16)


def _dev_layout(piece):
    """[TPC, D] fp16 shard piece -> [128, NG, NJ, GT] device layout."""
    return np.ascontiguousarray(
        piece.reshape(NG, GT, NJ, 128).transpose(3, 0, 2, 1))


def _prep_inputs(x, W, b):
    bf = ml_dtypes.bfloat16
    xf = np.ascontiguousarray(np.asarray(x).reshape(TOK, D)).astype(np.float32)
    xh, xm = _split_fp16(xf)
    Wf = np.asarray(W, dtype=np.float32)
    wH, wM = _split_fp16(Wf)
    wH = wH.reshape(NJ, 128, E).transpose(1, 0, 2)
    wM = wM.reshape(NJ, 128, E).transpose(1, 0, 2)
    wHM = np.ascontiguousarray(np.concatenate([wH, wM], axis=2))
    bias = np.asarray(b, dtype=np.float32).reshape(1, E)
    ident128 = np.eye(128, dtype=np.float32).astype(bf)
    ident64 = np.eye(64, dtype=np.float32)
    onescol = np.ones((128, 1), dtype=np.float32)

    in_maps = []
    for c in range(NCORES):
        pm = np.zeros((NCORES, 1), dtype=np.float32)
        pm[:c] = 1.0
        sl = slice(c * TPC, (c + 1) * TPC)
        in_maps.append({
            "xh": _dev_layout(xh[sl]),
            "xm": _dev_layout(xm[sl]),
            "wHM": wHM, "bias": bias, "ident128": ident128,
            "ident64": ident64, "onescol": onescol,
            "prevmask": pm,
        })
    return in_maps


def run(x, W, b, trace=False, trace_cores=None):
    nc = _get_program()
    in_maps = _prep_inputs(x, W, b)
    kw = {}
    if trace_cores is not None:
        kw["trace_cores"] = trace_cores
    res = bass_utils.run_bass_kernel_spmd(
        nc, in_maps, core_ids=list(range(NCORES)), trace=trace, **kw)
    shards = [
        np.transpose(res.results[c]["out"], (1, 0, 2)).reshape(TPC, E)
        for c in range(NCORES)
    ]
    routed = np.concatenate(shards, axis=0).reshape(B, S, E).astype(np.float32)
    aux_loss = np.float32(res.results[0]["aux"][0, 0])
    return (routed, aux_loss), res


def kernel(x, W, b):
    (routed, aux_loss), _ = run(x, W, b, trace=False)
    return routed, aux_loss
